# revision 1
# baseline (speedup 1.0000x reference)
"""Distributed Trainium2 Bass kernel for the ChebConv(K=3) GNN autoencoder.

Strategy (8 NeuronCores, SPMD):
  - Nodes padded to 8 shards x 12544 (98 blocks of 128). Core c owns dst
    shard c.
  - Each propagation out = A @ t is computed as: dma_gather of t rows by
    edge src (4 sub-gathers by src "quarter" so indices fit int16), then
    per dst-block one-hot matmuls on the TensorEngine:
        q_block[128 dst, D] += M_tile[128 slots, 128 dst].T @ y[128 slots, D]
    where M carries the (weighted) edge->dst-local one-hot, built on host.
  - Edge slots are laid out per (dst-block, src-quarter) cell with STATIC
    budgets (max over cores) so the program is identical on all cores.
  - BatchNorm is algebraically folded: all affine applications commute with
    A, so only raw aggregates (q = A p, r = A q) cross cores. BN statistics
    ride the AllGather payload as extra rows; weights are re-scaled on
    device each layer:
        out = p (a(W0-W2)) + q (a W1) + r (2a W2)
              + 1 (b^T(W0-W2) + bias) + s (b^T W1) + 2As (b^T W2)
    with (a, b) the previous layer's BN affine, s = A 1, As = A s.
  - Cross-core traffic: one AllGather of q and one of p' (pre-BN
    activations + stats) per layer, plus one tiny final AllReduce.
"""

import numpy as np
import ml_dtypes

N_CORES = 8
QW = 2          # shards per src-quarter window (int16 index limit)
NQ = N_CORES // QW   # 4 quarter windows

BF16 = ml_dtypes.bfloat16


# --------------------------------------------------------------------------
# host-side graph preprocessing
# --------------------------------------------------------------------------

class Plan:
    pass


def preprocess_graph(edge_index, n_nodes, gslot=2048, mch=16):
    """Build the static slot/pair layout and per-core gather/M data."""
    p = Plan()
    p.N = n_nodes
    src = edge_index[0].astype(np.int64)
    dst = edge_index[1].astype(np.int64)

    # edge weights exactly as the reference computes them (fp32)
    no_loop = (src != dst)
    deg = np.bincount(src, weights=no_loop.astype(np.float64), minlength=n_nodes)
    deg = deg.astype(np.float32)
    dinv = np.where(deg > 0, (1.0 / np.sqrt(np.maximum(deg, 1.0))).astype(np.float32),
                    np.float32(0.0)).astype(np.float32)
    w_all = (-dinv[src] * dinv[dst]).astype(np.float32)
    w_all[~no_loop] = 0.0

    keep = no_loop
    src, dst, w = src[keep], dst[keep], w_all[keep]
    p.E = len(src)

    # s = A 1 ;  As = A s   (per-node structure vectors)
    s_vec = np.bincount(dst, weights=w.astype(np.float64), minlength=n_nodes)
    s_vec = s_vec.astype(np.float32)
    as_vec = np.bincount(dst, weights=(w * s_vec[src]).astype(np.float64),
                         minlength=n_nodes).astype(np.float32)
    p.s_vec, p.as_vec = s_vec, as_vec

    # shard geometry
    SH = -(-n_nodes // (N_CORES * 128)) * 128      # shard rows (mult of 128)
    p.SH = SH
    p.NB = SH // 128                               # blocks per shard
    p.NPAD = SH * N_CORES
    p.PSTRIDE = SH + 4                             # p-tensor shard stride (4 stats rows)
    p.QWIN = QW * SH                               # rows per quarter window (q-variant)
    assert p.QWIN <= 32768 - 128
    assert QW * p.PSTRIDE <= 32768 - 128

    core = dst // SH
    blk = (dst % SH) // 128
    dloc = (dst % SH) % 128
    qtr = src // p.QWIN

    # per-cell counts and static budgets (max over cores)
    cnt = np.zeros((N_CORES, p.NB, NQ), np.int64)
    np.add.at(cnt, (core, blk, qtr), 1)
    B = cnt.max(axis=0)                            # [NB, NQ]
    p.budgets = B

    # quarter stream offsets
    S = np.zeros((p.NB, NQ), np.int64)
    for q in range(NQ):
        S[:, q] = np.concatenate([[0], np.cumsum(B[:, q])[:-1]])
    Lq = B.sum(axis=0)                             # stream length per quarter
    p.S = S
    p.gslot = gslot
    p.ninstr = [int(-(-int(l) // gslot)) for l in Lq]
    p.lqpad = [n * gslot for n in p.ninstr]
    p.qbase = np.concatenate([[0], np.cumsum(p.lqpad)]).astype(np.int64)
    p.nslots = int(p.qbase[-1])

    # static pair schedule: (b, q, col_in_quarter_stream)
    pairs = []
    pair_of = {}
    for b in range(p.NB):
        for q in range(NQ):
            if B[b, q] == 0:
                continue
            c0 = S[b, q] // 128
            c1 = (S[b, q] + B[b, q] - 1) // 128
            for c in range(c0, c1 + 1):
                pair_of[(b, q, c)] = len(pairs)
                pairs.append((b, q, c))
    p.pairs = pairs
    p.npairs = len(pairs)
    p.mch = mch
    p.nmchunks = -(-p.npairs // mch)

    # ---- per-core slot assignment ----
    # slot position for each edge: S[b,q] + running index within its cell
    order = np.lexsort((src, blk, qtr, core))      # edges grouped by (core, qtr, blk)
    src_o, dst_o, w_o = src[order], dst[order], w[order]
    core_o, blk_o, qtr_o, dloc_o = core[order], blk[order], qtr[order], dloc[order]
    # running index within (core, qtr, blk) group
    gid = ((core_o * NQ + qtr_o) * p.NB + blk_o)
    idx_in_cell = np.arange(len(gid)) - np.concatenate(
        [[0], np.cumsum(np.bincount(gid, minlength=N_CORES * NQ * p.NB))]
    )[gid]
    slot = p.qbase[qtr_o] + S[blk_o, qtr_o] + idx_in_cell

    p.slots = {}
    for c in range(N_CORES):
        m = core_o == c
        p.slots[c] = (slot[m], src_o[m], w_o[m], dloc_o[m], blk_o[m], qtr_o[m])
    return p


def build_idx_arrays(p):
    """int16 gather index arrays per core, for the q-layout (stride SH) and
    p-layout (stride PSTRIDE, skipping stats rows). [128, nslots/16]."""
    idxq = {}
    idxp = {}
    for c in range(N_CORES):
        slot, src_o, _, _, _, qtr_o = p.slots[c]
        iq = np.zeros(p.nslots, np.int16)
        ip = np.zeros(p.nslots, np.int16)
        sh = src_o // p.SH
        loc = src_o % p.SH
        iq[slot] = ((sh - qtr_o * QW) * p.SH + loc).astype(np.int16)
        ip[slot] = ((sh - qtr_o * QW) * p.PSTRIDE + loc).astype(np.int16)
        # pad slots keep idx 0 (valid row; weight 0 in M)
        idxq[c] = wrap_idx(iq)
        idxp[c] = wrap_idx(ip)
    return idxq, idxp


def wrap_idx(flat):
    """[nslots] -> [128, nslots//16] in the dma_gather wrapped+replicated layout."""
    n = len(flat)
    assert n % 16 == 0
    a = flat.reshape(n // 16, 16).T                # [16, n/16]
    return np.tile(a, (8, 1)).copy()


def build_m_stream(p):
    """Per-core M tiles [npairs, 128, 128] bf16, packed in chunk layout
    [nmchunks, 128, mch*128]."""
    out = {}
    pair_index = {}
    for i, (b, q, c) in enumerate(p.pairs):
        pair_index[(b, q, c)] = i
    for core in range(N_CORES):
        slot, _, w_o, dloc_o, blk_o, qtr_o = p.slots[core]
        M = np.zeros((p.npairs, 128, 128), np.float32)
        qrel = slot - p.qbase[qtr_o]
        col = qrel // 128
        row = qrel % 128
        pi = np.array([pair_index[(b, q, c)]
                       for b, q, c in zip(blk_o, qtr_o, col)], np.int64)
        M[pi, row, dloc_o] = w_o
        M = M.astype(BF16)
        # chunk layout: chunk k partition p holds tiles [k*mch..): row p
        padn = p.nmchunks * p.mch
        Mp = np.zeros((padn, 128, 128), BF16)
        Mp[:p.npairs] = M
        out[core] = (Mp.reshape(p.nmchunks, p.mch, 128, 128)
                       .transpose(0, 2, 1, 3)
                       .reshape(p.nmchunks, 128, p.mch * 128).copy())
    return out


# --------------------------------------------------------------------------
# host-side golden check of the slot/M construction (numpy only)
# --------------------------------------------------------------------------

def host_check_agg(p, t_full):
    """Compute q = A t via the slot/M machinery in numpy; return [NPAD, D]."""
    D = t_full.shape[1]
    q = np.zeros((N_CORES, p.SH, D), np.float32)
    idxq, _ = build_idx_arrays(p)
    mstream = build_m_stream(p)
    t_pad = np.zeros((p.NPAD, D), np.float32)
    t_pad[:p.N] = t_full
    for c in range(N_CORES):
        # unwrap idx
        a = idxq[c][:16, :]                        # [16, nslots/16]
        flat = a.T.reshape(-1)                     # [nslots]
        y = np.zeros((p.nslots, D), np.float32)
        for q_i in range(NQ):
            lo, hi = p.qbase[q_i], p.qbase[q_i + 1]
            base = q_i * QW * p.SH
            y[lo:hi] = t_pad[base + flat[lo:hi].astype(np.int64)]
        Mc = mstream[c].reshape(p.nmchunks, 128, p.mch, 128) \
                       .transpose(0, 2, 1, 3).reshape(-1, 128, 128)
        for i, (b, q_i, col) in enumerate(p.pairs):
            ycol = y[p.qbase[q_i] + col * 128: p.qbase[q_i] + (col + 1) * 128]
            q[c, b * 128:(b + 1) * 128] += Mc[i].astype(np.float32).T @ ycol
    return q.reshape(N_CORES * p.SH, D)


# --------------------------------------------------------------------------
# device program
# --------------------------------------------------------------------------

def build_nc(p, HID):
    import concourse.bacc as bacc
    import concourse.mybir as mybir
    import concourse.tile as tile
    from concourse.masks import make_identity

    dt = mybir.dt
    AF = mybir.ActivationFunctionType
    SH, NB, NPAD, PSTRIDE = p.SH, p.NB, p.NPAD, p.PSTRIDE
    GS = p.gslot
    NCOL = GS // 128          # y columns per gather instruction
    INV_N = 1.0 / p.N
    EPS = 1e-5

    nc = bacc.Bacc("TRN2", target_bir_lowering=False, debug=False,
                   num_devices=N_CORES, num_swdge_queues=4)

    def inp(name, shape, dtype):
        return nc.dram_tensor(name, shape, dtype, kind="ExternalInput")

    # ---- inputs ----
    idxq_t = inp("idxq", [128, p.nslots // 16], dt.int16)
    idxp_t = inp("idxp", [128, p.nslots // 16], dt.int16)
    mstr_t = inp("mstream", [p.nmchunks, 128, p.mch * 128], dt.bfloat16)
    h0full_t = inp("h0full", [NPAD, 128], dt.bfloat16)
    h0sh_t = inp("h0shard", [SH, 128], dt.bfloat16)
    ucoef_t = inp("ucoef", [3, SH], dt.bfloat16)
    maskc_t = inp("maskcol", [128, NB], dt.bfloat16)
    wp_t, bias_t, g_t, be_t, al_t = {}, {}, {}, {}, {}
    ldims = {1: (16, HID), 2: (HID, HID), 3: (HID, HID), 4: (HID, HID // 2)}
    for l in (1, 2, 3, 4):
        din, dout = ldims[l]
        nk = -(-3 * din // 128)
        wp_t[l] = inp(f"wp{l}", [nk, 128, dout], dt.bfloat16)
        bias_t[l] = inp(f"bias{l}", [1, dout], dt.float32)
        g_t[l] = inp(f"g{l}", [1, dout], dt.float32)
        be_t[l] = inp(f"be{l}", [1, dout], dt.float32)
        al_t[l] = inp(f"al{l}", [1, 1], dt.float32)

    out_t = nc.dram_tensor("out_shard", [SH, HID // 2], dt.float32,
                           kind="ExternalOutput")

    # ---- internal DRAM ----
    pfull = nc.dram_tensor("pfull", [N_CORES * PSTRIDE, HID], dt.bfloat16,
                           addr_space="Shared")
    qfull = nc.dram_tensor("qfull", [NPAD, HID], dt.bfloat16,
                           addr_space="Shared")
    agq_in = [nc.dram_tensor(f"agq_in{i}", [SH, HID], dt.bfloat16)
              for i in (0, 1)]
    agp_in = [nc.dram_tensor(f"agp_in{i}", [PSTRIDE, HID], dt.bfloat16)
              for i in (0, 1)]
    rtmp = nc.dram_tensor("rtmp", [SH, HID], dt.bfloat16)
    p4tmp = nc.dram_tensor("p4tmp", [SH, HID // 2], dt.bfloat16)
    arst_in = nc.dram_tensor("arst_in", [1, HID], dt.float32)
    arst_out = nc.dram_tensor("arst_out", [1, HID], dt.float32,
                              addr_space="Shared")
    RG = [list(range(N_CORES))]

    with tile.TileContext(nc) as tc:
        with (
            tc.tile_pool(name="resident", bufs=1) as rp,
            tc.tile_pool(name="y0", bufs=2) as yp0,
            tc.tile_pool(name="y1", bufs=2) as yp1,
            tc.tile_pool(name="y2", bufs=2) as yp2,
            tc.tile_pool(name="y3", bufs=2) as yp3,
            tc.tile_pool(name="mchunk", bufs=2) as mp,
            tc.tile_pool(name="blkio", bufs=3) as bp,
            tc.tile_pool(name="misc", bufs=1) as msp,
            tc.tile_pool(name="aggps", bufs=2, space="PSUM") as aggps,
            tc.tile_pool(name="trps", bufs=2, space="PSUM") as trps,
            tc.tile_pool(name="denseps", bufs=2, space="PSUM") as dps,
            tc.tile_pool(name="statsps", bufs=1, space="PSUM") as sps,
        ):
            ypools = [yp0, yp1, yp2, yp3]

            # resident tiles
            idxq = rp.tile([128, p.nslots // 16], dt.int16, tag="idxq")
            idxp = rp.tile([128, p.nslots // 16], dt.int16, tag="idxp")
            nc.sync.dma_start(out=idxq[:], in_=idxq_t[:])
            nc.sync.dma_start(out=idxp[:], in_=idxp_t[:])
            maskc = rp.tile([128, NB], dt.bfloat16, tag="maskc")
            nc.sync.dma_start(out=maskc[:], in_=maskc_t[:])
            ident = rp.tile([128, 128], dt.float32, tag="ident")
            make_identity(nc, ident[:])
            ident16 = rp.tile([128, 128], dt.bfloat16, tag="ident16")
            nc.vector.tensor_copy(ident16[:], ident[:])
            onesf = rp.tile([1, 128], dt.float32, tag="onesf")
            nc.vector.memset(onesf[:], 1.0)
            ones8 = rp.tile([8, 1], dt.float32, tag="ones8")
            nc.vector.memset(ones8[:], 1.0)

            def emit_gathers(src_ap_fn, idx_tile, elem, estep, tag):
                """Issue all gather instructions for one propagation.
                Returns list of per-quarter lists of y tiles (3D APs)."""
                ytiles = [[] for _ in range(NQ)]
                for q in range(NQ):
                    for k in range(p.ninstr[q]):
                        t = ypools[q].tile([128, NCOL * elem], dt.bfloat16,
                                           tag="y")
                        t3 = t[:].rearrange("a (n e) -> a n e", e=elem)
                        base = int(p.qbase[q]) + k * GS
                        nc.gpsimd.dma_gather(
                            out_ap=t3,
                            in_ap=src_ap_fn(q),
                            idxs_ap=idx_tile[:, base // 16:(base + GS) // 16],
                            num_idxs=GS, num_idxs_reg=GS,
                            elem_size=elem, elem_step=estep,
                            single_packet=False, queue_num=q,
                        )
                        ytiles[q].append(t3)
                return ytiles

            mchunk_state = {"tile": None, "idx": -1}

            def mtile(i):
                ch = i // p.mch
                if mchunk_state["idx"] != ch:
                    t = mp.tile([128, p.mch * 128], dt.bfloat16, tag="mch")
                    nc.sync.dma_start(out=t[:], in_=mstr_t[ch])
                    mchunk_state["tile"] = t
                    mchunk_state["idx"] = ch
                j = i % p.mch
                return mchunk_state["tile"][:, j * 128:(j + 1) * 128]

            pair_ranges = []       # pairs grouped by block
            cur = 0
            for b in range(NB):
                n = sum(1 for (bb, _, _) in p.pairs if bb == b)
                pair_ranges.append((cur, cur + n))
                cur += n

            def emit_agg(ytiles, elem, out_writer):
                """One propagation: per-block one-hot matmuls."""
                mchunk_state["idx"] = -1
                for b in range(NB):
                    lo, hi = pair_ranges[b]
                    ps = aggps.tile([128, elem], dt.float32, space="PSUM",
                                    tag="aggps")
                    for j in range(lo, hi):
                        _, q, col = p.pairs[j]
                        k, lc = col // NCOL, col % NCOL
                        nc.tensor.matmul(ps[:], lhsT=mtile(j),
                                         rhs=ytiles[q][k][:, lc, :],
                                         start=(j == lo), stop=(j == hi - 1))
                    out_writer(b, ps)

            def copy_out(dst_dram, elem):
                def w(b, ps):
                    sb = bp.tile([128, elem], dt.bfloat16, tag="aggout")
                    nc.scalar.copy(sb[:], ps[:])
                    nc.sync.dma_start(
                        out=dst_dram[b * 128:(b + 1) * 128, :elem], in_=sb[:])
                return w

            def transpose_into(dst_ap, src_ap, rows, use_vector):
                """PE-transpose src [128, rows] bf16 -> dst_ap ([rows, 128])."""
                tp = trps.tile([128, 128], dt.bfloat16, space="PSUM", tag="trp")
                nc.tensor.transpose(out=tp[:rows, :], in_=src_ap,
                                    identity=ident16[:])
                if use_vector:
                    nc.vector.tensor_copy(dst_ap, tp[:rows, :])
                else:
                    nc.scalar.copy(dst_ap, tp[:rows, :])

            def bcast_row(row_ap, width, out_dtype, tag):
                """[1, width] f32 -> [128, width] via ones matmul."""
                ps = trps.tile([128, width], dt.float32, space="PSUM", tag="trp")
                nc.tensor.matmul(ps[:], lhsT=onesf[:, :128], rhs=row_ap,
                                 start=True, stop=True)
                sb = msp.tile([128, width], out_dtype, tag=tag, name=tag)
                nc.vector.tensor_copy(sb[:], ps[:])
                return sb

            def transpose_row(row_ap, width, tag):
                """[1, width] f32 -> [128, nch] f32 (chunk h in column h)."""
                nch = -(-width // 128)
                col = msp.tile([128, nch], dt.float32, tag=tag, name=tag)
                for h in range(nch):
                    wdt = min(128, width - h * 128)
                    tp = trps.tile([128, 128], dt.float32, space="PSUM",
                                   tag="trp")
                    nc.tensor.transpose(
                        out=tp[:wdt, :1],
                        in_=row_ap[:, h * 128:h * 128 + wdt],
                        identity=ident[:1, :1])
                    nc.vector.tensor_copy(col[:wdt, h:h + 1], tp[:wdt, :1])
                return col

            # ---- per-layer emission ----
            def qwin_ap(tensor, q, stride_rows, width):
                base = q * QW * stride_rows
                return tensor[base:base + QW * stride_rows, :width]

            def emit_prelude(l, din):
                """Compute BN affine (a,b) of layer l-1 from pfull stats,
                fold weights, build V. Returns dict of tiles."""
                nk = -(-3 * din // 128)
                wraw = rp.tile([128, nk * ldims[l][1]], dt.bfloat16,
                               tag="wraw")
                for k in range(nk):
                    nc.sync.dma_start(
                        out=wraw[:, k * ldims[l][1]:(k + 1) * ldims[l][1]],
                        in_=wp_t[l][k])
                dout = ldims[l][1]
                if l == 1:
                    wfold = wraw
                    at_col = None
                    # V = [bias; 0; 0]
                    vt = msp.tile([3, dout], dt.bfloat16, tag="vt")
                    nc.vector.memset(vt[:], 0.0)
                    vrow0 = msp.tile([1, dout], dt.bfloat16, tag="vrow",
                                     name="vrow0l1")
                    nc.vector.tensor_copy(vrow0[:], bias_t_sb[l][:])
                    nc.sync.dma_start(out=vt[0:1, :], in_=vrow0[:])
                else:
                    # stats of p_{l-1}: [8 shards, 512] f32
                    st = msp.tile([8, 2 * din], dt.float32, tag="stt")
                    for s in range(N_CORES):
                        src = pfull[s * PSTRIDE + SH: s * PSTRIDE + SH + 4, :] \
                            .bitcast(dt.float32).rearrange("a b -> (a b)")
                        nc.sync.dma_start(out=st[s:s + 1, :], in_=src)
                    ssum = trps.tile([1, 2 * din], dt.float32, space="PSUM",
                                     tag="trp")
                    nc.tensor.matmul(ssum[:], lhsT=ones8[:], rhs=st[:],
                                     start=True, stop=True)
                    m1 = msp.tile([1, din], dt.float32, tag="m1")
                    m2 = msp.tile([1, din], dt.float32, tag="m2")
                    nc.vector.tensor_scalar_mul(m1[:], ssum[:, :din],
                                                float(INV_N))
                    nc.vector.tensor_scalar_mul(m2[:], ssum[:, din:],
                                                float(INV_N))
                    var = msp.tile([1, din], dt.float32, tag="var")
                    nc.vector.tensor_tensor(out=var[:], in0=m1[:], in1=m1[:],
                                            op=mybir.AluOpType.mult)
                    nc.vector.tensor_tensor(out=var[:], in0=m2[:], in1=var[:],
                                            op=mybir.AluOpType.subtract)
                    nc.vector.tensor_scalar_add(var[:], var[:], float(EPS))
                    rstd = msp.tile([1, din], dt.float32, tag="rstd")
                    nc.scalar.sqrt(rstd[:], var[:])
                    nc.vector.reciprocal(rstd[:], rstd[:])
                    arow = msp.tile([1, din], dt.float32, tag="arow")
                    nc.vector.tensor_tensor(out=arow[:], in0=g_t_sb[l - 1][:],
                                            in1=rstd[:],
                                            op=mybir.AluOpType.mult)
                    brow = msp.tile([1, din], dt.float32, tag="brow")
                    nc.vector.tensor_tensor(out=brow[:], in0=m1[:],
                                            in1=arow[:],
                                            op=mybir.AluOpType.mult)
                    nc.vector.tensor_tensor(out=brow[:], in0=be_t_sb[l - 1][:],
                                            in1=brow[:],
                                            op=mybir.AluOpType.subtract)
                    at_col = transpose_row(arow[:], din, "atcol")
                    bt_col = transpose_row(brow[:], din, "btcol")
                    bt16 = msp.tile([128, 2], dt.bfloat16, tag="bt16")
                    nc.vector.tensor_copy(bt16[:], bt_col[:])
                    # fold: wfold[k] = wraw[k] * a[feat of chunk k]
                    wfold = rp.tile([128, nk * dout], dt.bfloat16, tag="wfold")
                    for k in range(nk):
                        nc.vector.tensor_scalar_mul(
                            wfold[:, k * dout:(k + 1) * dout],
                            wraw[:, k * dout:(k + 1) * dout],
                            at_col[:, (k % 2):(k % 2) + 1])
                    # V rows: b^T part_i (+ bias on row0)
                    vt = msp.tile([3, dout], dt.bfloat16, tag="vt")
                    for part in range(3):
                        vps = trps.tile([1, dout], dt.float32, space="PSUM",
                                        tag="trp")
                        for h in range(2):
                            k = part * 2 + h
                            nc.tensor.matmul(
                                vps[:], lhsT=bt16[:, h:h + 1],
                                rhs=wraw[:, k * dout:(k + 1) * dout],
                                start=(h == 0), stop=(h == 1))
                        vrow = msp.tile([1, dout], dt.bfloat16, tag="vrow",
                                        name=f"vrow{part}")
                        if part == 0:
                            nc.vector.tensor_tensor(
                                out=vrow[:], in0=vps[:], in1=bias_t_sb[l][:],
                                op=mybir.AluOpType.add)
                        else:
                            nc.vector.tensor_copy(vrow[:], vps[:])
                        nc.sync.dma_start(out=vt[part:part + 1, :],
                                          in_=vrow[:])
                albc = bcast_row(al_t_sb[l][:], 1, dt.float32, "albc")
                return dict(wfold=wfold, vt=vt, albc=albc, nk=nk)

            def emit_dense(l, din, dout, pin_shard, pin_width, qin, rin, pr,
                           pout_dram):
                """Phase C: fused dense matmul + PReLU + stats."""
                nkc = 128 if l > 1 else 16      # feat rows per transpose
                s1ps = sps.tile([1, dout], dt.float32, space="PSUM", tag="s1")
                s2ps = sps.tile([1, dout], dt.float32, space="PSUM", tag="s2")
                for b in range(NB):
                    rows = slice(b * 128, (b + 1) * 128)
                    blks = []
                    for t_i, (dram, width) in enumerate(
                            ((pin_shard, pin_width), (qin, din), (rin, din))):
                        t = bp.tile([128, din], dt.bfloat16, tag=f"cin{t_i}")
                        nc.sync.dma_start(out=t[:, :din],
                                          in_=dram[rows, :din])
                        blks.append(t)
                    if l == 1:
                        actt = bp.tile([96, 128], dt.bfloat16, tag="actt1")
                        nc.vector.memset(actt[:], 0.0)
                        for t_i in range(3):
                            transpose_into(actt[t_i * 32:t_i * 32 + 16, :],
                                           blks[t_i][:, :16], 16, t_i % 2 == 0)
                    else:
                        actt = bp.tile([128, 6 * 128], dt.bfloat16, tag="actt")
                        for t_i in range(3):
                            for h in range(2):
                                k = t_i * 2 + h
                                transpose_into(
                                    actt[:, k * 128:(k + 1) * 128],
                                    blks[t_i][:, h * 128:(h + 1) * 128],
                                    128, k % 2 == 0)
                    ps2 = dps.tile([128, dout], dt.float32, space="PSUM",
                                   tag="dps")
                    if l == 1:
                        nc.tensor.matmul(ps2[:], lhsT=actt[:96, :],
                                         rhs=pr["wfold"][:96, :dout],
                                         start=True, stop=False)
                    else:
                        for k in range(pr["nk"]):
                            nc.tensor.matmul(
                                ps2[:], lhsT=actt[:, k * 128:(k + 1) * 128],
                                rhs=pr["wfold"][:, k * dout:(k + 1) * dout],
                                start=(k == 0), stop=False)
                    ublk = bp.tile([3, 128], dt.bfloat16, tag="ublk")
                    nc.sync.dma_start(out=ublk[:], in_=ucoef_t[:, rows])
                    nc.tensor.matmul(ps2[:], lhsT=ublk[:],
                                     rhs=pr["vt"][:], start=False, stop=True)
                    # PReLU(x) = max(x, a*x) for slope a in [0, 1)
                    ax = bp.tile([128, dout], dt.float32, tag="ax")
                    nc.scalar.activation(ax[:], ps2[:], AF.Copy,
                                         scale=pr["albc"][:])
                    pp = bp.tile([128, dout], dt.bfloat16, tag="pp")
                    nc.vector.tensor_tensor(out=pp[:], in0=ps2[:], in1=ax[:],
                                            op=mybir.AluOpType.max)
                    sq = bp.tile([128, dout], dt.bfloat16, tag="sq")
                    nc.vector.tensor_tensor(out=sq[:], in0=pp[:], in1=pp[:],
                                            op=mybir.AluOpType.mult)
                    nc.tensor.matmul(s1ps[:], lhsT=maskc[:, b:b + 1],
                                     rhs=pp[:], start=(b == 0),
                                     stop=(b == NB - 1))
                    nc.tensor.matmul(s2ps[:], lhsT=maskc[:, b:b + 1],
                                     rhs=sq[:], start=(b == 0),
                                     stop=(b == NB - 1))
                    nc.sync.dma_start(out=pout_dram[rows, :dout], in_=pp[:])
                return s1ps, s2ps

            def write_stats(dst_dram, s1ps, s2ps, dout):
                s1 = msp.tile([1, dout], dt.float32, tag="s1sb")
                s2 = msp.tile([1, dout], dt.float32, tag="s2sb")
                nc.scalar.copy(s1[:], s1ps[:])
                nc.scalar.copy(s2[:], s2ps[:])
                dview = dst_dram[SH:SH + 4, :].bitcast(dt.float32) \
                    .rearrange("a b -> (a b)")
                nc.sync.dma_start(out=dview[:dout], in_=s1[:])
                nc.sync.dma_start(out=dview[256:256 + dout], in_=s2[:])

            # small per-layer rows resident in SBUF
            bias_t_sb, g_t_sb, be_t_sb, al_t_sb = {}, {}, {}, {}
            for l in (1, 2, 3, 4):
                dout = ldims[l][1]
                bias_t_sb[l] = rp.tile([1, dout], dt.float32, tag=f"biassb{l}", name=f"biassb{l}")
                nc.sync.dma_start(out=bias_t_sb[l][:], in_=bias_t[l][:])
                g_t_sb[l] = rp.tile([1, dout], dt.float32, tag=f"gsb{l}", name=f"gsb{l}")
                nc.sync.dma_start(out=g_t_sb[l][:], in_=g_t[l][:])
                be_t_sb[l] = rp.tile([1, dout], dt.float32, tag=f"besb{l}", name=f"besb{l}")
                nc.sync.dma_start(out=be_t_sb[l][:], in_=be_t[l][:])
                al_t_sb[l] = rp.tile([1, 1], dt.float32, tag=f"alsb{l}", name=f"alsb{l}")
                nc.sync.dma_start(out=al_t_sb[l][:], in_=al_t[l][:])

            # ---- layers ----
            for l in (1, 2, 3, 4):
                din, dout = ldims[l]
                agq = agq_in[l % 2]
                agp_prev = agp_in[(l - 1) % 2]
                agp_cur = agp_in[l % 2]
                if l == 1:
                    elem1, step1 = 128, 128
                    src1 = lambda q: qwin_ap(h0full_t, q, SH, 128)
                    idx1 = idxq
                    pin_shard, pin_w = h0sh_t, 128
                else:
                    elem1, step1 = HID, HID
                    src1 = lambda q: qwin_ap(pfull, q, PSTRIDE, HID)
                    idx1 = idxp
                    pin_shard, pin_w = agp_prev, HID
                elem2 = 128 if l == 1 else HID

                # phase A: q = A p
                yt = emit_gathers(src1, idx1, elem1, step1, tag="A")
                emit_agg(yt, elem1, copy_out(agq, elem1))
                nc.gpsimd.collective_compute(
                    "AllGather", mybir.AluOpType.bypass, replica_groups=RG,
                    ins=[agq[:]], outs=[qfull[:]])

                # phase B: r = A q
                yt = emit_gathers(lambda q, e=elem2: qwin_ap(qfull, q, SH, e),
                                  idxq, elem2, HID, tag="B")
                emit_agg(yt, elem2, copy_out(rtmp, elem2))

                # phase C
                pr = emit_prelude(l, din)
                pout = agp_cur if l < 4 else p4tmp
                s1ps, s2ps = emit_dense(l, din, dout, pin_shard, pin_w,
                                        agq, rtmp, pr, pout)
                if l < 4:
                    write_stats(agp_cur, s1ps, s2ps, dout)
                    nc.gpsimd.collective_compute(
                        "AllGather", mybir.AluOpType.bypass, replica_groups=RG,
                        ins=[agp_cur[:]], outs=[pfull[:]])
                else:
                    s1 = msp.tile([1, dout], dt.float32, tag="s1sb")
                    s2 = msp.tile([1, dout], dt.float32, tag="s2sb")
                    nc.scalar.copy(s1[:], s1ps[:])
                    nc.scalar.copy(s2[:], s2ps[:])
                    nc.sync.dma_start(out=arst_in[:, :dout], in_=s1[:])
                    nc.sync.dma_start(out=arst_in[:, dout:], in_=s2[:])
                    nc.gpsimd.collective_compute(
                        "AllReduce", mybir.AluOpType.add, replica_groups=RG,
                        ins=[arst_in[:]], outs=[arst_out[:]])

            # ---- final BN affine of layer 4 ----
            dout = ldims[4][1]
            stf = msp.tile([1, 2 * dout], dt.float32, tag="stf")
            nc.sync.dma_start(out=stf[:], in_=arst_out[:])
            m1f = msp.tile([1, dout], dt.float32, tag="m1f")
            m2f = msp.tile([1, dout], dt.float32, tag="m2f")
            nc.vector.tensor_scalar_mul(m1f[:], stf[:, :dout], float(INV_N))
            nc.vector.tensor_scalar_mul(m2f[:], stf[:, dout:], float(INV_N))
            varf = msp.tile([1, dout], dt.float32, tag="varf")
            nc.vector.tensor_tensor(out=varf[:], in0=m1f[:], in1=m1f[:],
                                    op=mybir.AluOpType.mult)
            nc.vector.tensor_tensor(out=varf[:], in0=m2f[:], in1=varf[:],
                                    op=mybir.AluOpType.subtract)
            nc.vector.tensor_scalar_add(varf[:], varf[:], float(EPS))
            rstdf = msp.tile([1, dout], dt.float32, tag="rstdf")
            nc.scalar.sqrt(rstdf[:], varf[:])
            nc.vector.reciprocal(rstdf[:], rstdf[:])
            a4row = msp.tile([1, dout], dt.float32, tag="a4row")
            nc.vector.tensor_tensor(out=a4row[:], in0=g_t_sb[4][:],
                                    in1=rstdf[:], op=mybir.AluOpType.mult)
            b4row = msp.tile([1, dout], dt.float32, tag="b4row")
            nc.vector.tensor_tensor(out=b4row[:], in0=m1f[:],
                                    in1=a4row[:], op=mybir.AluOpType.mult)
            nc.vector.tensor_tensor(out=b4row[:], in0=be_t_sb[4][:],
                                    in1=b4row[:], op=mybir.AluOpType.subtract)
            a4bc = bcast_row(a4row[:], dout, dt.float32, "a4bc")
            b4bc = bcast_row(b4row[:], dout, dt.float32, "b4bc")
            for b in range(NB):
                rows = slice(b * 128, (b + 1) * 128)
                t = bp.tile([128, dout], dt.bfloat16, tag="fin")
                nc.sync.dma_start(out=t[:], in_=p4tmp[rows, :])
                o = bp.tile([128, dout], dt.float32, tag="fout")
                nc.vector.tensor_tensor(out=o[:], in0=t[:], in1=a4bc[:],
                                        op=mybir.AluOpType.mult)
                nc.vector.tensor_tensor(out=o[:], in0=o[:], in1=b4bc[:],
                                        op=mybir.AluOpType.add)
                nc.sync.dma_start(out=out_t[rows, :], in_=o[:])

    nc.finalize()
    return nc


# --------------------------------------------------------------------------
# public entry point
# --------------------------------------------------------------------------

_CACHE = {}
_SIM = False
_TRACE = False
_LAST_EXEC_NS = None


def _run_sim(nc, in_maps):
    from concourse.bass_interp import MultiCoreSim
    sim = MultiCoreSim(nc, N_CORES, require_finite=False, require_nnan=False)
    for c in range(N_CORES):
        for k, v in in_maps[c].items():
            sim.cores[c].tensor(k)[:] = v
    sim.simulate()
    return [{"out_shard": sim.cores[c].tensor("out_shard").copy()}
            for c in range(N_CORES)]


def kernel(x, pos, normals, edge_index,
           W1, b1, a1, g1, be1, W2, b2, a2, g2, be2,
           W3, b3, a3, g3, be3, W4, b4, a4, g4, be4):
    return _kernel_impl(x, pos, normals, edge_index,
                        (W1, b1, a1, g1, be1), (W2, b2, a2, g2, be2),
                        (W3, b3, a3, g3, be3), (W4, b4, a4, g4, be4))


def _kernel_impl(x, pos, normals, edge_index, L1, L2, L3, L4):
    from concourse.bass_utils import run_bass_kernel_spmd

    x = np.asarray(x)
    HID = np.asarray(L1[0]).shape[2]
    N = x.shape[0]
    key = (N, edge_index.shape[1], HID, hash(edge_index.tobytes()))
    if key not in _CACHE:
        p = preprocess_graph(np.asarray(edge_index), N)
        nc = build_nc(p, HID)
        idxq, idxp = build_idx_arrays(p)
        mstream = build_m_stream(p)
        _CACHE[key] = (p, nc, idxq, idxp, mstream)
    p, nc, idxq, idxp, mstream = _CACHE[key]

    # host data prep
    h0 = np.concatenate([x, np.asarray(pos), np.asarray(normals)], axis=1) \
           .astype(np.float32)                         # [N, 9]
    h0p = np.zeros((p.NPAD, 128), BF16)
    h0p[:N, :h0.shape[1]] = h0.astype(BF16)

    in_maps = []
    for c in range(N_CORES):
        im = {
            "idxq": idxq[c], "idxp": idxp[c], "mstream": mstream[c],
            "h0full": h0p,
            "h0shard": h0p[c * p.SH:(c + 1) * p.SH].copy(),
        }
        uc = np.zeros((3, p.SH), np.float32)
        lo, hi = c * p.SH, min((c + 1) * p.SH, N)
        n_real = hi - lo
        uc[0, :] = 1.0
        if n_real > 0:
            uc[1, :n_real] = p.s_vec[lo:hi]
            uc[2, :n_real] = p.as_vec[lo:hi]
        im["ucoef"] = uc.astype(BF16)
        mc = np.zeros((p.SH,), np.float32)
        if n_real > 0:
            mc[:n_real] = 1.0
        im["maskcol"] = mc.reshape(p.NB, 128).T.astype(BF16).copy()
        for l, (W, b, a, g, be) in enumerate((L1, L2, L3, L4), start=1):
            W = np.asarray(W).astype(np.float32)       # [3, din, dout]
            din, dout = W.shape[1], W.shape[2]
            if l == 1:
                wchunks = np.zeros((1, 128, dout), np.float32)
                wchunks[0, 0:din] = W[0] - W[2]
                wchunks[0, 32:32 + din] = W[1]
                wchunks[0, 64:64 + din] = 2.0 * W[2]
            else:
                flat = np.concatenate([W[0] - W[2], W[1], 2.0 * W[2]], axis=0)
                nk = 3 * din // 128
                wchunks = flat.reshape(nk, 128, dout)
            im[f"wp{l}"] = wchunks.astype(BF16)
            im[f"bias{l}"] = np.asarray(b, np.float32).reshape(1, dout)
            im[f"g{l}"] = np.asarray(g, np.float32).reshape(1, dout)
            im[f"be{l}"] = np.asarray(be, np.float32).reshape(1, dout)
            im[f"al{l}"] = np.asarray(a, np.float32).reshape(1, 1)
        in_maps.append(im)

    global _LAST_EXEC_NS
    if _SIM:
        results = _run_sim(nc, in_maps)
    else:
        res = run_bass_kernel_spmd(
            nc, in_maps, core_ids=list(range(N_CORES)), trace=_TRACE)
        _LAST_EXEC_NS = res.exec_time_ns
        results = res.results
    shards = [results[c]["out_shard"] for c in range(N_CORES)]
    return np.concatenate(shards, axis=0)[:N].astype(np.float32)



# revision 10
# speedup vs baseline: 1.3893x; 1.3893x over previous
"""Distributed Trainium2 Bass kernel for the ChebConv(K=3) GNN autoencoder.

v3 strategy (8 NeuronCores, SPMD), derived from baseline trace analysis
(7.25ms: DMA ~47% busy, PE 2.46ms busy, collectives 1.5ms serial,
gather descriptor-gen 2.25ms serial on GpSimd):

  - z-scheme per layer:  out = h(W0-W2) + L z + bias  with
    z = h W1 + 2 (L h) W2  computed LOCALLY between the two
    propagations.  Only two AllGathers per layer (z~ and p~'), no
    r/q DRAM round-trips (phase A psum feeds dense1 directly).
  - One-hot aggregation: edge weights w = -dinv_src*dinv_dst are
    factored out of the scatter matrix M:  all propagated tensors are
    stored pre-scaled by dinv (source side), M entries are exactly 1.0
    (stored fp8e4m3, partially SBUF-resident), and the dst-side scale
    -dinv rides the psum->SBUF copy as a per-partition scalar.
  - Uniform [8*SH, W] layout for every gather source -> a single int16
    index array serves all 8 propagations.
  - BN stats travel through a tiny per-layer AllReduce issued BEFORE
    the big p~ AllGather, so the next layer's BN-affine / weight folds
    and the o_p pass (p-dependent dense part, staged via DRAM) execute
    during the collective windows.
  - Layer 1 keeps the classic two-propagation form (h0 is only 9 wide);
    layer 4's z is 128 wide (halves its second propagation).
"""

import numpy as np
import ml_dtypes

N_CORES = 8
QW = 2                 # shards per src-quarter window (int16 index limit)
NQ = N_CORES // QW     # 4 quarter windows
GSLOT = 2048
MCH = 16               # M tiles per streamed chunk
R_CHUNKS = 26          # resident M chunks (R_CHUNKS*MCH tiles stay in SBUF)

BF16 = ml_dtypes.bfloat16
FP8 = ml_dtypes.float8_e4m3


# --------------------------------------------------------------------------
# host-side graph preprocessing
# --------------------------------------------------------------------------

class Plan:
    pass


def preprocess_graph(edge_index, n_nodes, gslot=GSLOT, mch=MCH):
    """Build the static slot/pair layout and per-core gather/M data."""
    p = Plan()
    p.N = n_nodes
    src = edge_index[0].astype(np.int64)
    dst = edge_index[1].astype(np.int64)

    no_loop = (src != dst)
    deg = np.bincount(src, weights=no_loop.astype(np.float64), minlength=n_nodes)
    deg = deg.astype(np.float32)
    dinv = np.where(deg > 0, (1.0 / np.sqrt(np.maximum(deg, 1.0))).astype(np.float32),
                    np.float32(0.0)).astype(np.float32)
    p.dinv = dinv

    keep = no_loop
    src, dst = src[keep], dst[keep]
    p.E = len(src)

    # s = L 1 (per-node structure vector)
    w = (-dinv[src] * dinv[dst]).astype(np.float32)
    s_vec = np.bincount(dst, weights=w.astype(np.float64), minlength=n_nodes)
    p.s_vec = s_vec.astype(np.float32)

    # shard geometry
    SH = -(-n_nodes // (N_CORES * 128)) * 128
    p.SH = SH
    p.NB = SH // 128
    p.NPAD = SH * N_CORES
    p.QWIN = QW * SH
    assert p.QWIN <= 32768 - 128

    core = dst // SH
    blk = (dst % SH) // 128
    dloc = (dst % SH) % 128
    qtr = src // p.QWIN

    # per-cell counts and static budgets (max over cores)
    cnt = np.zeros((N_CORES, p.NB, NQ), np.int64)
    np.add.at(cnt, (core, blk, qtr), 1)
    B = cnt.max(axis=0)                            # [NB, NQ]
    p.budgets = B

    # quarter stream offsets
    S = np.zeros((p.NB, NQ), np.int64)
    for q in range(NQ):
        S[:, q] = np.concatenate([[0], np.cumsum(B[:, q])[:-1]])
    Lq = B.sum(axis=0)
    p.S = S
    p.gslot = gslot
    # gather instruction sizes: full gslot except a 128-rounded tail
    p.gsizes = []
    for q in range(NQ):
        L = int(Lq[q])
        n = max(1, -(-L // gslot))
        tail = L - gslot * (n - 1)
        tail = max(128, -(-tail // 128) * 128)
        p.gsizes.append([gslot] * (n - 1) + [tail])
    p.lqpad = [sum(s) for s in p.gsizes]
    p.qbase = np.concatenate([[0], np.cumsum(p.lqpad)]).astype(np.int64)
    p.nslots = int(p.qbase[-1])
    # column -> (gather instr, col-within-instr) per quarter
    p.colmap = []
    for q in range(NQ):
        cm = []
        for k, gs in enumerate(p.gsizes[q]):
            cm += [(k, c) for c in range(gs // 128)]
        p.colmap.append(cm)

    # static pair schedule: (b, q, col_in_quarter_stream)
    pairs = []
    for b in range(p.NB):
        for q in range(NQ):
            if B[b, q] == 0:
                continue
            c0 = S[b, q] // 128
            c1 = (S[b, q] + B[b, q] - 1) // 128
            for c in range(c0, c1 + 1):
                pairs.append((b, q, c))
    p.pairs = pairs
    p.npairs = len(pairs)
    p.mch = mch
    p.nmchunks = -(-p.npairs // mch)
    p.rchunks = min(R_CHUNKS, p.nmchunks)
    p.rtiles = p.rchunks * mch

    pair_ranges = []
    cur = 0
    for b in range(p.NB):
        n = sum(1 for (bb, _, _) in pairs if bb == b)
        pair_ranges.append((cur, cur + n))
        cur += n
    p.pair_ranges = pair_ranges

    # ---- per-core slot assignment ----
    order = np.lexsort((src, blk, qtr, core))
    src_o, dst_o = src[order], dst[order]
    core_o, blk_o, qtr_o, dloc_o = core[order], blk[order], qtr[order], dloc[order]
    gid = ((core_o * NQ + qtr_o) * p.NB + blk_o)
    idx_in_cell = np.arange(len(gid)) - np.concatenate(
        [[0], np.cumsum(np.bincount(gid, minlength=N_CORES * NQ * p.NB))]
    )[gid]
    slot = p.qbase[qtr_o] + S[blk_o, qtr_o] + idx_in_cell

    p.slots = {}
    for c in range(N_CORES):
        m = core_o == c
        p.slots[c] = (slot[m], src_o[m], dloc_o[m], blk_o[m], qtr_o[m])
    return p


def build_idx_array(p):
    """Single int16 gather index array per core, stride SH. [128, nslots/16]."""
    out = {}
    for c in range(N_CORES):
        slot, src_o, _, _, qtr_o = p.slots[c]
        iq = np.zeros(p.nslots, np.int16)
        sh = src_o // p.SH
        loc = src_o % p.SH
        iq[slot] = ((sh - qtr_o * QW) * p.SH + loc).astype(np.int16)
        out[c] = wrap_idx(iq)
    return out


def wrap_idx(flat):
    n = len(flat)
    assert n % 16 == 0
    a = flat.reshape(n // 16, 16).T
    return np.tile(a, (8, 1)).copy()


def build_m_stream(p):
    """Per-core one-hot M tiles (fp8, value 1.0) in chunk layout.
    Returns (mres [128, rtiles*128], mstr [nmchunks-rchunks, 128, mch*128])."""
    pair_index = {}
    for i, (b, q, c) in enumerate(p.pairs):
        pair_index[(b, q, c)] = i
    out = {}
    for core in range(N_CORES):
        slot, _, dloc_o, blk_o, qtr_o = p.slots[core]
        M = np.zeros((p.nmchunks * p.mch, 128, 128), FP8)
        qrel = slot - p.qbase[qtr_o]
        col = qrel // 128
        row = qrel % 128
        pi = np.array([pair_index[(b, q, c)]
                       for b, q, c in zip(blk_o, qtr_o, col)], np.int64)
        M[pi, row, dloc_o] = FP8(1.0)
        chunks = (M.reshape(p.nmchunks, p.mch, 128, 128)
                    .transpose(0, 2, 1, 3)
                    .reshape(p.nmchunks, 128, p.mch * 128))
        mres = (chunks[:p.rchunks].transpose(1, 0, 2)
                .reshape(128, p.rchunks * p.mch * 128).copy())
        mstr = chunks[p.rchunks:].copy()
        if mstr.shape[0] == 0:
            mstr = np.zeros((1, 128, p.mch * 128), FP8)
        out[core] = (mres, mstr)
    return out


# --------------------------------------------------------------------------
# host-side golden check of the slot/M construction (numpy only)
# --------------------------------------------------------------------------

def host_check_agg(p, t_full, width):
    """Compute S @ (dinv*t) via the slot/M machinery in numpy.
    Returns [NPAD, width] per-dst-core result BEFORE the -dinv dst scale."""
    idxs = build_idx_array(p)
    ms = build_m_stream(p)
    t_pad = np.zeros((p.NPAD, width), np.float32)
    t_pad[:p.N] = (p.dinv[:, None] * t_full[:, :width]).astype(BF16).astype(np.float32)
    out = np.zeros((N_CORES, p.SH, width), np.float32)
    for c in range(N_CORES):
        a = idxs[c][:16, :]
        flat = a.T.reshape(-1)
        y = np.zeros((p.nslots, width), np.float32)
        for q_i in range(NQ):
            lo, hi = int(p.qbase[q_i]), int(p.qbase[q_i + 1])
            base = q_i * QW * p.SH
            y[lo:hi] = t_pad[base + flat[lo:hi].astype(np.int64)]
        mres, mstr = ms[c]
        allm = np.concatenate(
            [mres.reshape(128, p.rchunks, p.mch, 128).transpose(1, 2, 0, 3)
                 .reshape(-1, 128, 128),
             mstr.reshape(-1, 128, p.mch, 128).transpose(0, 2, 1, 3)
                 .reshape(-1, 128, 128)], axis=0)
        for i, (b, q_i, col) in enumerate(p.pairs):
            ycol = y[int(p.qbase[q_i]) + col * 128:
                     int(p.qbase[q_i]) + (col + 1) * 128]
            out[c, b * 128:(b + 1) * 128] += \
                allm[i].astype(np.float32).T @ ycol
    return out.reshape(p.NPAD, width)


# --------------------------------------------------------------------------
# device program
# --------------------------------------------------------------------------

def build_nc(p, HID):
    import concourse.bacc as bacc
    import concourse.mybir as mybir
    import concourse.tile as tile
    from concourse.masks import make_identity

    dt = mybir.dt
    AF = mybir.ActivationFunctionType
    OP = mybir.AluOpType
    SH, NB, NPAD = p.SH, p.NB, p.NPAD
    INV_N = 1.0 / p.N
    EPS = 1e-5
    H2 = HID // 2
    RG = [list(range(N_CORES))]

    nc = bacc.Bacc("TRN2", target_bir_lowering=False, debug=False,
                   num_devices=N_CORES, num_swdge_queues=4)

    def inp(name, shape, dtype):
        return nc.dram_tensor(name, shape, dtype, kind="ExternalInput")

    # ---- inputs ----
    idx_t = inp("idx", [128, p.nslots // 16], dt.int16)
    mres_t = inp("mres", [128, p.rtiles * 128], dt.float8e4)
    nstream = max(1, p.nmchunks - p.rchunks)
    mstr_t = inp("mstr", [nstream, 128, p.mch * 128], dt.float8e4)
    h0full_t = inp("h0full", [NPAD, 128], dt.bfloat16)
    h0sh16_t = inp("h0sh16", [SH, 16], dt.bfloat16)
    dpos_t = inp("dpos", [128, NB], dt.float32)
    dneg_t = inp("dneg", [128, NB], dt.float32)
    dnegsq_t = inp("dnegsq", [128, NB], dt.float32)
    maskc_t = inp("maskc", [128, NB], dt.bfloat16)
    ub_t = inp("ub", [2, SH], dt.bfloat16)
    wf1a_t = inp("wf1a", [16, HID], dt.bfloat16)
    wf1b_t = inp("wf1b", [64, HID], dt.bfloat16)
    wz_t, ww_t, wop_t = {}, {}, {}
    bias_t, g_t, be_t, al_t = {}, {}, {}, {}
    ldout = {1: HID, 2: HID, 3: HID, 4: H2}
    for l in (2, 3, 4):
        d = ldout[l]
        wz_t[l] = inp(f"wz{l}", [2, 128, d], dt.bfloat16)
        ww_t[l] = inp(f"ww{l}", [2, 128, d], dt.bfloat16)
        wop_t[l] = inp(f"wop{l}", [2, 128, d], dt.bfloat16)
    for l in (1, 2, 3, 4):
        d = ldout[l]
        bias_t[l] = inp(f"bias{l}", [1, d], dt.float32)
        g_t[l] = inp(f"g{l}", [1, d], dt.float32)
        be_t[l] = inp(f"be{l}", [1, d], dt.float32)
        al_t[l] = inp(f"al{l}", [1, 1], dt.float32)

    out_t = nc.dram_tensor("out_shard", [SH, H2], dt.float32,
                           kind="ExternalOutput")

    # ---- internal DRAM ----
    q1ag = nc.dram_tensor("q1ag", [SH, 128], dt.bfloat16)
    q1full = nc.dram_tensor("q1full", [NPAD, 128], dt.bfloat16,
                            addr_space="Shared")
    zag = nc.dram_tensor("zag", [SH, HID], dt.bfloat16)
    zfull = nc.dram_tensor("zfull", [NPAD, HID], dt.bfloat16,
                           addr_space="Shared")
    z4ag = nc.dram_tensor("z4ag", [SH, H2], dt.bfloat16)
    z4full = nc.dram_tensor("z4full", [NPAD, H2], dt.bfloat16,
                            addr_space="Shared")
    pag = nc.dram_tensor("pag", [SH, HID], dt.bfloat16)
    pf = [nc.dram_tensor(f"pf{i}", [NPAD, HID], dt.bfloat16,
                         addr_space="Shared") for i in (0, 1)]
    agp = [nc.dram_tensor(f"agp{i}", [SH, HID], dt.bfloat16) for i in (0, 1)]
    opd = nc.dram_tensor("opd", [SH, HID], dt.bfloat16)
    p4tmp = nc.dram_tensor("p4tmp", [SH, H2], dt.bfloat16)
    ar_in = [nc.dram_tensor(f"ar_in{i}", [1, 2 * HID], dt.float32)
             for i in (0, 1)]
    ar_out = [nc.dram_tensor(f"ar_out{i}", [1, 2 * HID], dt.float32,
                             addr_space="Shared") for i in (0, 1)]
    ar4_in = nc.dram_tensor("ar4_in", [1, 2 * H2], dt.float32)
    ar4_out = nc.dram_tensor("ar4_out", [1, 2 * H2], dt.float32,
                             addr_space="Shared")

    with tile.TileContext(nc) as tc:
        with (
            tc.tile_pool(name="resident", bufs=1) as rp,
            tc.tile_pool(name="y0", bufs=2) as yp0,
            tc.tile_pool(name="y1", bufs=2) as yp1,
            tc.tile_pool(name="y2", bufs=2) as yp2,
            tc.tile_pool(name="y3", bufs=2) as yp3,
            tc.tile_pool(name="mchunk", bufs=2) as mp,
            tc.tile_pool(name="blkio", bufs=2) as bp,
            tc.tile_pool(name="wfold", bufs=1) as wp,
            tc.tile_pool(name="misc", bufs=1) as msp,
            tc.tile_pool(name="aggps", bufs=2, space="PSUM") as aggps,
            tc.tile_pool(name="trps", bufs=2, space="PSUM") as trps,
            tc.tile_pool(name="denseps", bufs=3, space="PSUM") as dps,
            tc.tile_pool(name="statsps", bufs=1, space="PSUM") as sps,
        ):
            ypools = [yp0, yp1, yp2, yp3]

            # ---- resident tiles ----
            idx = rp.tile([128, p.nslots // 16], dt.int16, tag="idx")
            nc.sync.dma_start(out=idx[:], in_=idx_t[:])
            mres = rp.tile([128, p.rtiles * 128], dt.float8e4, tag="mres")
            nc.sync.dma_start(out=mres[:], in_=mres_t[:])
            maskc = rp.tile([128, NB], dt.bfloat16, tag="maskc")
            nc.sync.dma_start(out=maskc[:], in_=maskc_t[:])
            dpos = rp.tile([128, NB], dt.float32, tag="dpos")
            nc.sync.dma_start(out=dpos[:], in_=dpos_t[:])
            dneg = rp.tile([128, NB], dt.float32, tag="dneg")
            nc.sync.dma_start(out=dneg[:], in_=dneg_t[:])
            dnegsq = rp.tile([128, NB], dt.float32, tag="dnegsq")
            nc.sync.dma_start(out=dnegsq[:], in_=dnegsq_t[:])
            h0r = rp.tile([128, NB * 16], dt.bfloat16, tag="h0r")
            nc.sync.dma_start(
                out=h0r[:].rearrange("p (b f) -> p b f", f=16),
                in_=h0sh16_t[:].rearrange("(b p) f -> p b f", p=128))
            q1r = rp.tile([128, NB * 16], dt.bfloat16, tag="q1r")
            wf1a = rp.tile([16, HID], dt.bfloat16, tag="wf1a")
            nc.sync.dma_start(out=wf1a[:], in_=wf1a_t[:])
            wf1b = rp.tile([64, HID], dt.bfloat16, tag="wf1b")
            nc.sync.dma_start(out=wf1b[:], in_=wf1b_t[:])
            wzr, wwr, wopr = {}, {}, {}
            for l in (2, 3, 4):
                d = ldout[l]
                for nm, tt_, store in (("wz", wz_t, wzr), ("ww", ww_t, wwr),
                                       ("wop", wop_t, wopr)):
                    t = rp.tile([128, 2 * d], dt.bfloat16, tag=f"{nm}{l}")
                    nc.sync.dma_start(
                        out=t[:].rearrange("p (k d) -> p k d", k=2),
                        in_=tt_[l][:].rearrange("k p d -> p k d"))
                    store[l] = t
            bias_sb, g_sb, be_sb, al_sb = {}, {}, {}, {}
            for l in (1, 2, 3, 4):
                d = ldout[l]
                bias_sb[l] = rp.tile([1, d], dt.float32, tag=f"bias{l}", name=f"bias_sb{l}")
                nc.sync.dma_start(out=bias_sb[l][:], in_=bias_t[l][:])
                g_sb[l] = rp.tile([1, d], dt.float32, tag=f"g{l}", name=f"g_sb{l}")
                nc.sync.dma_start(out=g_sb[l][:], in_=g_t[l][:])
                be_sb[l] = rp.tile([1, d], dt.float32, tag=f"be{l}", name=f"be_sb{l}")
                nc.sync.dma_start(out=be_sb[l][:], in_=be_t[l][:])
                al_sb[l] = rp.tile([1, 1], dt.float32, tag=f"al{l}", name=f"al_sb{l}")
                nc.sync.dma_start(out=al_sb[l][:], in_=al_t[l][:])
            ident = rp.tile([128, 128], dt.float32, tag="ident")
            make_identity(nc, ident[:])
            ident16 = rp.tile([128, 128], dt.bfloat16, tag="ident16")
            nc.vector.tensor_copy(ident16[:], ident[:])
            onesf = rp.tile([1, 128], dt.float32, tag="onesf")
            nc.vector.memset(onesf[:], 1.0)
            ones1 = rp.tile([1, 128], dt.bfloat16, tag="ones1")
            nc.vector.memset(ones1[:], 1.0)
            vc1 = rp.tile([1, HID], dt.bfloat16, tag="vc1")
            nc.vector.tensor_copy(vc1[:], bias_sb[1][:])

            # ---- helpers ----
            def emit_gathers(src_fn, width):
                ytiles = [[] for _ in range(NQ)]
                for q in range(NQ):
                    off = 0
                    for gs in p.gsizes[q]:
                        ncol = gs // 128
                        t = ypools[q].tile([128, 16 * HID], dt.bfloat16,
                                           tag="y")
                        t3 = t[:].rearrange("a (n e) -> a n e",
                                            e=width)[:, :ncol, :]
                        base = int(p.qbase[q]) + off
                        nc.gpsimd.dma_gather(
                            out_ap=t3,
                            in_ap=src_fn(q),
                            idxs_ap=idx[:, base // 16:(base + gs) // 16],
                            num_idxs=gs, num_idxs_reg=gs,
                            elem_size=width, elem_step=width,
                            single_packet=False, queue_num=q,
                        )
                        ytiles[q].append(t3)
                        off += gs
                return ytiles

            mstate = {"tile": None, "idx": -1}

            def mtile(j):
                if j < p.rtiles:
                    return mres[:, j * 128:(j + 1) * 128]
                ch = j // p.mch
                if mstate["idx"] != ch:
                    t = mp.tile([128, p.mch * 128], dt.float8e4, tag="mch")
                    nc.sync.dma_start(out=t[:], in_=mstr_t[ch - p.rchunks])
                    mstate["tile"] = t
                    mstate["idx"] = ch
                jj = j % p.mch
                return mstate["tile"][:, jj * 128:(jj + 1) * 128]

            def emit_agg(ytiles, free_w, writer):
                mstate["idx"] = -1
                for b in range(NB):
                    lo, hi = p.pair_ranges[b]
                    ps = aggps.tile([128, free_w], dt.float32, space="PSUM",
                                    tag="aggps")
                    for j in range(lo, hi):
                        _, q, c = p.pairs[j]
                        k, lc = p.colmap[q][c]
                        nc.tensor.matmul(ps[:], lhsT=mtile(j),
                                         rhs=ytiles[q][k][:, lc, :free_w],
                                         start=(j == lo), stop=(j == hi - 1))
                    writer(b, ps)

            def transpose_into(dst_ap, src_ap, rows, use_vector):
                tp = trps.tile([128, 128], dt.bfloat16, space="PSUM",
                               tag="trp")
                nc.tensor.transpose(out=tp[:rows, :], in_=src_ap,
                                    identity=ident16[:])
                if use_vector:
                    nc.vector.tensor_copy(dst_ap, tp[:rows, :])
                else:
                    nc.scalar.copy(dst_ap, tp[:rows, :])

            def bcast_row(row_ap, width, tag):
                ps = trps.tile([128, 128], dt.float32, space="PSUM", tag="trp")
                nc.tensor.matmul(ps[:, :width], lhsT=onesf[:, :128],
                                 rhs=row_ap, start=True, stop=True)
                sb = msp.tile([128, width], dt.float32, tag=tag, name=tag)
                nc.vector.tensor_copy(sb[:], ps[:, :width])
                return sb

            def transpose_row(row_ap, width, tag):
                nch = -(-width // 128)
                col = msp.tile([128, nch], dt.float32, tag=tag, name=tag)
                for h in range(nch):
                    wdt = min(128, width - h * 128)
                    tp = trps.tile([128, 128], dt.float32, space="PSUM",
                                   tag="trp")
                    nc.tensor.transpose(
                        out=tp[:wdt, :1],
                        in_=row_ap[:, h * 128:h * 128 + wdt],
                        identity=ident[:1, :1])
                    nc.vector.tensor_copy(col[:wdt, h:h + 1], tp[:wdt, :1])
                return col

            def qwin(tensor, q, width):
                base = q * QW * SH
                return tensor[base:base + QW * SH, :width]

            # ---- prelude: BN affine of layer l-1 + weight folds for l ----
            def emit_prelude(l):
                din, d = ldout[l - 1], ldout[l]
                stf = msp.tile([1, 2 * din], dt.float32, tag="stf")
                nc.sync.dma_start(out=stf[:],
                                  in_=ar_out[(l - 1) % 2][:, :2 * din])
                m1 = msp.tile([1, din], dt.float32, tag="m1")
                m2 = msp.tile([1, din], dt.float32, tag="m2")
                nc.vector.tensor_scalar_mul(m1[:], stf[:, :din], float(INV_N))
                nc.vector.tensor_scalar_mul(m2[:], stf[:, din:], float(INV_N))
                var = msp.tile([1, din], dt.float32, tag="var")
                nc.vector.tensor_tensor(out=var[:], in0=m1[:], in1=m1[:],
                                        op=OP.mult)
                nc.vector.tensor_tensor(out=var[:], in0=m2[:], in1=var[:],
                                        op=OP.subtract)
                nc.vector.tensor_scalar_add(var[:], var[:], float(EPS))
                rstd = msp.tile([1, din], dt.float32, tag="rstd")
                nc.scalar.sqrt(rstd[:], var[:])
                nc.vector.reciprocal(rstd[:], rstd[:])
                arow = msp.tile([1, din], dt.float32, tag="arow")
                nc.vector.tensor_tensor(out=arow[:], in0=g_sb[l - 1][:],
                                        in1=rstd[:], op=OP.mult)
                brow = msp.tile([1, din], dt.float32, tag="brow")
                nc.vector.tensor_tensor(out=brow[:], in0=m1[:], in1=arow[:],
                                        op=OP.mult)
                nc.vector.tensor_tensor(out=brow[:], in0=be_sb[l - 1][:],
                                        in1=brow[:], op=OP.subtract)
                at_col = transpose_row(arow[:], din, "atcol")
                bt_col = transpose_row(brow[:], din, "btcol")
                bt16 = msp.tile([128, 2], dt.bfloat16, tag="bt16")
                nc.vector.tensor_copy(bt16[:], bt_col[:])
                wzf = wp.tile([128, 2 * d], dt.bfloat16, tag="wzf")
                wwf = wp.tile([128, 2 * d], dt.bfloat16, tag="wwf")
                wopf = wp.tile([128, 2 * d], dt.bfloat16, tag="wopf")
                for k in range(2):
                    sl = slice(k * d, (k + 1) * d)
                    nc.vector.tensor_scalar_mul(wzf[:, sl], wzr[l][:, sl],
                                                at_col[:, k:k + 1])
                    nc.vector.tensor_scalar_mul(wwf[:, sl], wwr[l][:, sl],
                                                at_col[:, k:k + 1])
                    nc.vector.tensor_scalar_mul(wopf[:, sl], wopr[l][:, sl],
                                                at_col[:, k:k + 1])
                vz = msp.tile([2, d], dt.bfloat16, tag="vz", name=f"vz{l}")
                for r, raw in ((0, wzr[l]), (1, wwr[l])):
                    vps = trps.tile([1, d], dt.float32, space="PSUM",
                                    tag="trp")
                    for h in range(2):
                        nc.tensor.matmul(vps[:], lhsT=bt16[:, h:h + 1],
                                         rhs=raw[:, h * d:(h + 1) * d],
                                         start=(h == 0), stop=(h == 1))
                    vrow = msp.tile([1, d], dt.bfloat16, tag="vrow",
                                    name=f"vrow{l}_{r}")
                    nc.vector.tensor_copy(vrow[:], vps[:])
                    nc.sync.dma_start(out=vz[r:r + 1, :], in_=vrow[:])
                vps = trps.tile([1, d], dt.float32, space="PSUM",
                                tag="trp")
                for h in range(2):
                    nc.tensor.matmul(vps[:], lhsT=bt16[:, h:h + 1],
                                     rhs=wopr[l][:, h * d:(h + 1) * d],
                                     start=(h == 0), stop=(h == 1))
                vcf = msp.tile([1, d], dt.float32, tag="vcf")
                nc.vector.tensor_tensor(out=vcf[:], in0=vps[:],
                                        in1=bias_sb[l][:], op=OP.add)
                vcst = msp.tile([1, d], dt.bfloat16, tag="vcst",
                                name=f"vcst{l}")
                nc.vector.tensor_copy(vcst[:], vcf[:])
                albc = bcast_row(al_sb[l][:], 1, "albc")
                return dict(wzf=wzf, wwf=wwf, wopf=wopf, vz=vz, vcst=vcst,
                            albc=albc)

            # ---- o_p pass over a block range (l >= 2) ----
            def emit_op_pass(l, pr, agp_prev, b0, b1):
                d = ldout[l]
                for b in range(b0, b1):
                    rows = slice(b * 128, (b + 1) * 128)
                    pb = bp.tile([128, HID], dt.bfloat16, tag="op_pb")
                    nc.scalar.dma_start(out=pb[:], in_=agp_prev[rows, :])
                    opt = bp.tile([128, 256], dt.bfloat16, tag="op_t")
                    for h in range(2):
                        transpose_into(opt[:, h * 128:(h + 1) * 128],
                                       pb[:, h * 128:(h + 1) * 128],
                                       128, h == 0)
                    ops = dps.tile([128, d], dt.float32, space="PSUM",
                                   tag="dps")
                    for h in range(2):
                        nc.tensor.matmul(ops[:], lhsT=opt[:, h * 128:(h + 1) * 128],
                                         rhs=pr["wopf"][:, h * d:(h + 1) * d],
                                         start=(h == 0), stop=False)
                    nc.tensor.matmul(ops[:], lhsT=ones1[:],
                                     rhs=pr["vcst"][:], start=False, stop=True)
                    opsb = bp.tile([128, d], dt.bfloat16, tag="op_sb")
                    nc.vector.tensor_copy(opsb[:], ops[:])
                    nc.sync.dma_start(out=opd[rows, :d], in_=opsb[:])

            # o_p pass for layer 1 (h0 term + bias)
            def emit_op1_pass():
                for b in range(NB):
                    rows = slice(b * 128, (b + 1) * 128)
                    h0t = bp.tile([16, 128], dt.bfloat16, tag="h0t")
                    transpose_into(h0t[:], h0r[:, b * 16:(b + 1) * 16],
                                   16, True)
                    ops = dps.tile([128, HID], dt.float32, space="PSUM",
                                   tag="dps")
                    nc.tensor.matmul(ops[:], lhsT=h0t[:], rhs=wf1a[:],
                                     start=True, stop=False)
                    nc.tensor.matmul(ops[:], lhsT=ones1[:], rhs=vc1[:],
                                     start=False, stop=True)
                    opsb = bp.tile([128, HID], dt.bfloat16, tag="op_sb")
                    nc.vector.tensor_copy(opsb[:], ops[:])
                    nc.sync.dma_start(out=opd[rows, :], in_=opsb[:])

            # ---- dense2 tail shared by all layers ----
            def dense2_tail(l, b, x_ap, s12ps, albc, pp_dst, scaled):
                d = ldout[l]
                rows = slice(b * 128, (b + 1) * 128)
                ax = bp.tile([128, d], dt.float32, tag="ax")
                nc.scalar.activation(ax[:], x_ap, AF.Copy, scale=albc[:])
                ppsq = bp.tile([128, 2 * d], dt.bfloat16, tag="ppsq")
                nc.vector.tensor_tensor(out=ppsq[:, :d], in0=x_ap, in1=ax[:],
                                        op=OP.max)
                nc.vector.tensor_tensor(out=ppsq[:, d:], in0=ppsq[:, :d],
                                        in1=ppsq[:, :d], op=OP.mult)
                nc.tensor.matmul(s12ps[:], lhsT=maskc[:, b:b + 1],
                                 rhs=ppsq[:], start=(b == 0),
                                 stop=(b == NB - 1))
                nc.sync.dma_start(out=pp_dst[rows, :d], in_=ppsq[:, :d])
                if scaled is not None:
                    ptld = bp.tile([128, d], dt.bfloat16, tag="ptld")
                    nc.scalar.activation(ptld[:], ppsq[:, :d], AF.Copy,
                                         scale=dpos[:, b:b + 1])
                    nc.scalar.dma_start(out=scaled[rows, :d], in_=ptld[:])

            # ================= LAYER 1 =================
            albc1 = bcast_row(al_sb[1][:], 1, "albc1")

            # phase A1: q1 = L h0  (gather 128-wide, only 16 cols used)
            yt = emit_gathers(lambda q: qwin(h0full_t, q, 128), 128)

            def a1_writer(b, ps):
                rows = slice(b * 128, (b + 1) * 128)
                q1sb = bp.tile([128, 16], dt.bfloat16, tag="q1sb")
                nc.scalar.activation(q1sb[:], ps[:, :16], AF.Copy,
                                     scale=dnegsq[:, b:b + 1])
                nc.sync.dma_start(out=q1ag[rows, :16], in_=q1sb[:])
                nc.vector.tensor_scalar_mul(q1r[:, b * 16:(b + 1) * 16],
                                            ps[:, :16], dneg[:, b:b + 1])
            emit_agg(yt, 16, a1_writer)

            nc.gpsimd.collective_compute(
                "AllGather", OP.bypass, replica_groups=RG,
                ins=[q1ag[:]], outs=[q1full[:]])

            # o_p1 pass runs during AG(q1)
            emit_op1_pass()

            # phase B1 + dense2(1)
            yt = emit_gathers(lambda q: qwin(q1full, q, 128), 128)
            s12ps = sps.tile([1, 2 * HID], dt.float32, space="PSUM",
                             tag="s12")

            def b1_writer(b, ps):
                rows = slice(b * 128, (b + 1) * 128)
                r1sb = bp.tile([128, 16], dt.bfloat16, tag="r1sb")
                nc.vector.tensor_scalar_mul(r1sb[:], ps[:, :16],
                                            dneg[:, b:b + 1])
                actt = bp.tile([64, 128], dt.bfloat16, tag="actt1")
                nc.vector.memset(actt[:], 0.0)
                transpose_into(actt[0:16, :], q1r[:, b * 16:(b + 1) * 16],
                               16, True)
                transpose_into(actt[32:48, :], r1sb[:], 16, False)
                ps2 = dps.tile([128, HID], dt.float32, space="PSUM",
                               tag="dps")
                nc.tensor.matmul(ps2[:], lhsT=actt[:], rhs=wf1b[:],
                                 start=True, stop=True)
                opb = bp.tile([128, HID], dt.bfloat16, tag="opb")
                nc.sync.dma_start(out=opb[:], in_=opd[rows, :])
                x = bp.tile([128, HID], dt.float32, tag="x")
                nc.vector.tensor_tensor(out=x[:], in0=ps2[:], in1=opb[:],
                                        op=OP.add)
                dense2_tail(1, b, x[:], s12ps, albc1, agp[0], pag)
            emit_agg(yt, 16, b1_writer)

            s12sb = msp.tile([1, 2 * HID], dt.float32, tag="s12sb")
            nc.scalar.copy(s12sb[:], s12ps[:])
            nc.sync.dma_start(out=ar_in[1][:], in_=s12sb[:])
            nc.gpsimd.collective_compute(
                "AllReduce", OP.add, replica_groups=RG,
                ins=[ar_in[1][:]], outs=[ar_out[1][:]])
            nc.gpsimd.collective_compute(
                "AllGather", OP.bypass, replica_groups=RG,
                ins=[pag[:]], outs=[pf[0][:]])

            # ================= LAYERS 2..4 =================
            for l in (2, 3, 4):
                d = ldout[l]
                agp_prev = agp[l % 2]
                agp_cur = agp[(l + 1) % 2] if l < 4 else None
                pfull = pf[l % 2]
                half = NB // 2

                pr = emit_prelude(l)
                # o_p first half: runs during AG(p~ of layer l-1)
                emit_op_pass(l, pr, agp_prev, 0, half)

                # phase A(l): q = L p, fused with dense1 (z build)
                yt = emit_gathers(lambda q: qwin(pfull, q, HID), HID)
                zag_l = zag if l < 4 else z4ag

                def a_writer(b, ps, l=l, d=d, pr=pr, agp_prev=agp_prev,
                             zag_l=zag_l):
                    rows = slice(b * 128, (b + 1) * 128)
                    qsb = bp.tile([128, HID], dt.bfloat16, tag="qsb")
                    nc.scalar.activation(qsb[:], ps[:], AF.Copy,
                                         scale=dneg[:, b:b + 1])
                    pb = bp.tile([128, HID], dt.bfloat16, tag="d1_pb")
                    nc.sync.dma_start(out=pb[:], in_=agp_prev[rows, :])
                    actt = bp.tile([128, 4 * 128], dt.bfloat16, tag="actt")
                    for h in range(2):
                        transpose_into(actt[:, h * 128:(h + 1) * 128],
                                       pb[:, h * 128:(h + 1) * 128],
                                       128, h == 0)
                    for h in range(2):
                        transpose_into(actt[:, (2 + h) * 128:(3 + h) * 128],
                                       qsb[:, h * 128:(h + 1) * 128],
                                       128, h == 1)
                    zps = dps.tile([128, d], dt.float32, space="PSUM",
                                   tag="dps")
                    for h in range(2):
                        nc.tensor.matmul(
                            zps[:], lhsT=actt[:, h * 128:(h + 1) * 128],
                            rhs=pr["wzf"][:, h * d:(h + 1) * d],
                            start=(h == 0), stop=False)
                    for h in range(2):
                        nc.tensor.matmul(
                            zps[:], lhsT=actt[:, (2 + h) * 128:(3 + h) * 128],
                            rhs=pr["wwf"][:, h * d:(h + 1) * d],
                            start=False, stop=False)
                    ubb = bp.tile([2, 128], dt.bfloat16, tag="ubb")
                    nc.sync.dma_start(out=ubb[:], in_=ub_t[:, rows])
                    nc.tensor.matmul(zps[:], lhsT=ubb[:], rhs=pr["vz"][:],
                                     start=False, stop=True)
                    ztld = bp.tile([128, d], dt.bfloat16, tag="ztld")
                    nc.scalar.activation(ztld[:], zps[:], AF.Copy,
                                         scale=dpos[:, b:b + 1])
                    nc.sync.dma_start(out=zag_l[rows, :d], in_=ztld[:])
                emit_agg(yt, HID, a_writer)

                zfull_l = zfull if l < 4 else z4full
                nc.gpsimd.collective_compute(
                    "AllGather", OP.bypass, replica_groups=RG,
                    ins=[zag_l[:]], outs=[zfull_l[:]])

                # o_p second half: runs during AG(z)
                emit_op_pass(l, pr, agp_prev, half, NB)

                # phase B(l) + dense2(l)
                yt = emit_gathers(lambda q: qwin(zfull_l, q, d), d)
                s12ps = sps.tile([1, 2 * d], dt.float32, space="PSUM",
                                 tag="s12")
                pp_dst = agp_cur if l < 4 else p4tmp
                scaled = pag if l < 4 else None

                def b_writer(b, ps, l=l, d=d, s12ps=s12ps, pr=pr,
                             pp_dst=pp_dst, scaled=scaled):
                    rows = slice(b * 128, (b + 1) * 128)
                    opb = bp.tile([128, d], dt.bfloat16, tag="opb")
                    nc.sync.dma_start(out=opb[:], in_=opd[rows, :d])
                    x1 = bp.tile([128, d], dt.float32, tag="x1")
                    nc.scalar.activation(x1[:], ps[:], AF.Copy,
                                         scale=dneg[:, b:b + 1])
                    x = bp.tile([128, d], dt.float32, tag="x")
                    nc.vector.tensor_tensor(out=x[:], in0=x1[:], in1=opb[:],
                                            op=OP.add)
                    dense2_tail(l, b, x[:], s12ps, pr["albc"], pp_dst,
                                scaled)
                emit_agg(yt, d, b_writer)

                s12sb = msp.tile([1, 2 * d], dt.float32, tag="s12sb")
                nc.scalar.copy(s12sb[:], s12ps[:])
                if l < 4:
                    nc.sync.dma_start(out=ar_in[l % 2][:, :2 * d],
                                      in_=s12sb[:])
                    nc.gpsimd.collective_compute(
                        "AllReduce", OP.add, replica_groups=RG,
                        ins=[ar_in[l % 2][:]], outs=[ar_out[l % 2][:]])
                    nc.gpsimd.collective_compute(
                        "AllGather", OP.bypass, replica_groups=RG,
                        ins=[pag[:]], outs=[pf[(l + 1) % 2][:]])
                else:
                    nc.sync.dma_start(out=ar4_in[:], in_=s12sb[:])
                    nc.gpsimd.collective_compute(
                        "AllReduce", OP.add, replica_groups=RG,
                        ins=[ar4_in[:]], outs=[ar4_out[:]])

            # ---- final BN affine of layer 4 ----
            d = H2
            stf = msp.tile([1, 2 * d], dt.float32, tag="stf4")
            nc.sync.dma_start(out=stf[:], in_=ar4_out[:])
            m1f = msp.tile([1, d], dt.float32, tag="m1f")
            m2f = msp.tile([1, d], dt.float32, tag="m2f")
            nc.vector.tensor_scalar_mul(m1f[:], stf[:, :d], float(INV_N))
            nc.vector.tensor_scalar_mul(m2f[:], stf[:, d:], float(INV_N))
            varf = msp.tile([1, d], dt.float32, tag="varf")
            nc.vector.tensor_tensor(out=varf[:], in0=m1f[:], in1=m1f[:],
                                    op=OP.mult)
            nc.vector.tensor_tensor(out=varf[:], in0=m2f[:], in1=varf[:],
                                    op=OP.subtract)
            nc.vector.tensor_scalar_add(varf[:], varf[:], float(EPS))
            rstdf = msp.tile([1, d], dt.float32, tag="rstdf")
            nc.scalar.sqrt(rstdf[:], varf[:])
            nc.vector.reciprocal(rstdf[:], rstdf[:])
            a4row = msp.tile([1, d], dt.float32, tag="a4row")
            nc.vector.tensor_tensor(out=a4row[:], in0=g_sb[4][:],
                                    in1=rstdf[:], op=OP.mult)
            b4row = msp.tile([1, d], dt.float32, tag="b4row")
            nc.vector.tensor_tensor(out=b4row[:], in0=m1f[:], in1=a4row[:],
                                    op=OP.mult)
            nc.vector.tensor_tensor(out=b4row[:], in0=be_sb[4][:],
                                    in1=b4row[:], op=OP.subtract)
            a4bc = bcast_row(a4row[:], d, "a4bc")
            b4bc = bcast_row(b4row[:], d, "b4bc")
            for b in range(NB):
                rows = slice(b * 128, (b + 1) * 128)
                t = bp.tile([128, d], dt.bfloat16, tag="fin")
                nc.sync.dma_start(out=t[:], in_=p4tmp[rows, :])
                o = bp.tile([128, d], dt.float32, tag="fout")
                nc.vector.tensor_tensor(out=o[:], in0=t[:], in1=a4bc[:],
                                        op=OP.mult)
                nc.vector.tensor_tensor(out=o[:], in0=o[:], in1=b4bc[:],
                                        op=OP.add)
                nc.sync.dma_start(out=out_t[rows, :], in_=o[:])

    nc.finalize()
    return nc


# --------------------------------------------------------------------------
# public entry point
# --------------------------------------------------------------------------

_CACHE = {}
_SIM = False
_TRACE = False
_LAST_EXEC_NS = None


def _run_sim(nc, in_maps):
    from concourse.bass_interp import MultiCoreSim
    sim = MultiCoreSim(nc, N_CORES, require_finite=False, require_nnan=False)
    for c in range(N_CORES):
        for k, v in in_maps[c].items():
            sim.cores[c].tensor(k)[:] = v
    sim.simulate()
    return [{"out_shard": sim.cores[c].tensor("out_shard").copy()}
            for c in range(N_CORES)]


def kernel(x, pos, normals, edge_index,
           W1, b1, a1, g1, be1, W2, b2, a2, g2, be2,
           W3, b3, a3, g3, be3, W4, b4, a4, g4, be4):
    return _kernel_impl(x, pos, normals, edge_index,
                        (W1, b1, a1, g1, be1), (W2, b2, a2, g2, be2),
                        (W3, b3, a3, g3, be3), (W4, b4, a4, g4, be4))


def _kernel_impl(x, pos, normals, edge_index, L1, L2, L3, L4):
    from concourse.bass_utils import run_bass_kernel_spmd

    x = np.asarray(x)
    HID = np.asarray(L1[0]).shape[2]
    N = x.shape[0]
    key = (N, edge_index.shape[1], HID, hash(edge_index.tobytes()))
    if key not in _CACHE:
        p = preprocess_graph(np.asarray(edge_index), N)
        nc = build_nc(p, HID)
        idxs = build_idx_array(p)
        ms = build_m_stream(p)
        _CACHE[key] = (p, nc, idxs, ms)
    p, nc, idxs, ms = _CACHE[key]
    SH, NB = p.SH, p.NB
    H2 = HID // 2

    h0 = np.concatenate([x, np.asarray(pos), np.asarray(normals)], axis=1) \
           .astype(np.float32)                         # [N, 9]
    h0sc = (p.dinv[:, None] * h0).astype(BF16)
    h0p = np.zeros((p.NPAD, 128), BF16)
    h0p[:N, :h0.shape[1]] = h0sc
    h0r = np.zeros((p.NPAD, 16), BF16)
    h0r[:N, :h0.shape[1]] = h0.astype(BF16)
    dinv_pad = np.zeros(p.NPAD, np.float32)
    dinv_pad[:N] = p.dinv
    s_pad = np.zeros(p.NPAD, np.float32)
    s_pad[:N] = p.s_vec

    in_maps = []
    for c in range(N_CORES):
        mres, mstr = ms[c]
        im = {
            "idx": idxs[c], "mres": mres, "mstr": mstr,
            "h0full": h0p,
            "h0sh16": h0r[c * SH:(c + 1) * SH].copy(),
        }
        dshard = dinv_pad[c * SH:(c + 1) * SH].reshape(NB, 128).T
        im["dpos"] = dshard.astype(np.float32).copy()
        im["dneg"] = (-dshard).astype(np.float32).copy()
        im["dnegsq"] = (-dshard * dshard).astype(np.float32).copy()
        lo, hi = c * SH, min((c + 1) * SH, N)
        n_real = max(0, hi - lo)
        mc = np.zeros((SH,), np.float32)
        mc[:n_real] = 1.0
        im["maskc"] = mc.reshape(NB, 128).T.astype(BF16).copy()
        ubx = np.zeros((2, SH), np.float32)
        ubx[0, :] = 1.0
        ubx[1, :] = s_pad[c * SH:(c + 1) * SH]
        im["ub"] = ubx.astype(BF16)
        Ws = {}
        for l, (W, b, a, g, be) in enumerate((L1, L2, L3, L4), start=1):
            W = np.asarray(W).astype(np.float32)
            dout = W.shape[2]
            Ws[l] = W
            im[f"bias{l}"] = np.asarray(b, np.float32).reshape(1, dout)
            im[f"g{l}"] = np.asarray(g, np.float32).reshape(1, dout)
            im[f"be{l}"] = np.asarray(be, np.float32).reshape(1, dout)
            im[f"al{l}"] = np.asarray(a, np.float32).reshape(1, 1)
        # L1 weights
        W = Ws[1]
        wf1a = np.zeros((16, HID), np.float32)
        wf1a[:9] = W[0] - W[2]
        im["wf1a"] = wf1a.astype(BF16)
        wf1b = np.zeros((64, HID), np.float32)
        wf1b[0:9] = W[1]
        wf1b[32:41] = 2.0 * W[2]
        im["wf1b"] = wf1b.astype(BF16)
        for l in (2, 3, 4):
            W = Ws[l]
            dout = W.shape[2]
            im[f"wz{l}"] = W[1].reshape(2, 128, dout).astype(BF16)
            im[f"ww{l}"] = (2.0 * W[2]).reshape(2, 128, dout).astype(BF16)
            im[f"wop{l}"] = (W[0] - W[2]).reshape(2, 128, dout).astype(BF16)
        in_maps.append(im)

    global _LAST_EXEC_NS
    if _SIM:
        results = _run_sim(nc, in_maps)
    else:
        res = run_bass_kernel_spmd(
            nc, in_maps, core_ids=list(range(N_CORES)), trace=_TRACE)
        _LAST_EXEC_NS = res.exec_time_ns
        results = res.results
    shards = [results[c]["out_shard"] for c in range(N_CORES)]
    return np.concatenate(shards, axis=0)[:N].astype(np.float32)
